# revision 45
# baseline (speedup 1.0000x reference)
"""Multi-head attention (N=2, K=2048, DIN=1024, H=16, DOUT=64) on 8 TRN2 NeuronCores.

Sharding: data-parallel over batch N (cores 0-3 -> n=0, cores 4-7 -> n=1),
tensor-parallel over heads (4 heads per core). Each core computes its 4 heads'
attention plus the partial output projection over its head-feature slice of Wp;
the host sums the 4 partials per batch element and adds the output bias.

Schedule (final): the attention phase targets joint ScalarE(exp)/DVE/PE
saturation at ~2.4us per kt tile.  All non-attention PE work (pair-1 k
projection, both v projections, output projection) is injected into the
attention kt loops through a DEDICATED psum bank so it never perturbs the
S-tile rotation; each head's S psum is its own single-buffered 2-bank tag so
S(h,kt+1) waits only on exp(h,kt):

  opening: warmup burst (HAM ramp, overlaps xq DMA), proj q pair0 + pair1
           (xq only), k pair0 (xk), then attention starts.
  block (0,0): + v-proj pair0 (2 st-tiles/unit) just-in-time for lagged PV
  block (0,1): + k-proj pair1
  block (1,0): + v-proj pair1
  block (1,1): + output projection for q 0..1024
  tail:    last epilogues + output projection for q 1024..2048

PSUM (8 banks): S-h0 2, S-h1 2 (also rdb), pv accumulators 2, dn 1
(single-buffered: the previous block's den rows are DVE-copied out at kt 2
and the bank is re-allocated at kt 4), injection bank 1.

Per-core kernel structure (all matmuls bf16, fp32 PSUM accumulation):
  - host pre-transposes/casts activations to bf16 [DIN, SEQ].
  - q/k projected head-pair-wise to [dout(2 heads on partitions), seq];
    v projected per-pair to the natural [seq, dout] layout.
  - scores computed transposed (S.T = k_h @ q_h.T) so softmax probabilities
    are already in the layout the PV matmul needs; no max-subtraction
    (|S/8| < 10, exp safe in fp32); mask applied as 0/1 multiply AFTER exp.
  - PV matmuls lag the S/exp/mask stream by >=3 kt (ring of ptm tiles) and
    col-pack 2 heads per PSUM bank; softmax denominators accumulate via
    col-packed M=1 ones-matmuls (lagged) in the dn bank.
  - accumulators are DVE-zeroed and accumulating matmuls use start=False.
  - epilogue per qi half: 1/den via custom-DVE reciprocal on base-0 copies,
    bf16 cast, two col-packed K=1 outer products broadcast 1/den across both
    heads' 64 rows, one full-height multiply normalizes 128 x 512 at once.
  - epilogue work is spread over the NEXT block's kt slots (den copies kt 2,
    normalize kts 3/5) and the final dn/PV matmuls + pv drains are emitted
    inside the next block's first two kt slots (cross-block carry).
  - output tiles drain on DVE inside block (1,1), on ScalarE in the tail;
    output is written bf16 (halves the out-DMA).
"""

import numpy as np
import ml_dtypes

import concourse.bass as bass
import concourse.mybir as mybir
from concourse import bacc
from concourse.tile import TileContext

P = 128
SEQ = 2048
DIN = 1024
DOUT = 64
H = 16
N = 2
HPC = 4  # heads per core
NPAIR = 2  # head pairs per core
KSUB = DIN // P  # 8 contraction subtiles for projections
NKT = SEQ // P  # 16 seq_k tiles of 128
BF = mybir.dt.bfloat16
F32 = mybir.dt.float32
BF_NP = ml_dtypes.bfloat16

N_WARM = 88  # opening warmup matmuls (dependency-free, ramps HAM to 8/8)

_NC_CACHE = None


def build_bass():
    nc = bacc.Bacc()

    xq_d = nc.declare_dram_parameter("xqT", [DIN, SEQ], BF, isOutput=False)
    xk_d = nc.declare_dram_parameter("xkT", [DIN, SEQ], BF, isOutput=False)
    xv_d = nc.declare_dram_parameter("xvT", [DIN, SEQ], BF, isOutput=False)
    mk_d = nc.declare_dram_parameter("maskT", [SEQ, SEQ], BF, isOutput=False)
    wq_d = nc.declare_dram_parameter("wq", [DIN, HPC * DOUT], BF, isOutput=False)
    wk_d = nc.declare_dram_parameter("wk", [DIN, HPC * DOUT], BF, isOutput=False)
    wv_d = nc.declare_dram_parameter("wv", [DIN, HPC * DOUT], BF, isOutput=False)
    wp_d = nc.declare_dram_parameter("wp", [HPC * DOUT, DIN], BF, isOutput=False)
    bqp_d = nc.declare_dram_parameter("bqp", [P, NPAIR], F32, isOutput=False)
    bkp_d = nc.declare_dram_parameter("bkp", [P, NPAIR], F32, isOutput=False)
    bvr_d = nc.declare_dram_parameter("bvr", [P, HPC * DOUT], F32, isOutput=False)
    out_d = nc.declare_dram_parameter("out", [SEQ, DIN], BF, isOutput=True)

    ADD = mybir.AluOpType.add
    EXP = mybir.ActivationFunctionType.Exp

    with TileContext(nc) as tc:
        with (
            tc.tile_pool(name="const", bufs=1) as const,
            tc.tile_pool(name="xin", bufs=1) as xin,
            tc.tile_pool(name="proj", bufs=1) as proj,
            tc.tile_pool(name="maskp", bufs=3) as maskp,
            tc.tile_pool(name="ptp", bufs=2) as ptp,
            tc.tile_pool(name="epi", bufs=2) as epi,
            tc.tile_pool(name="ps_s", bufs=1, space="PSUM") as ps_s,
            tc.tile_pool(name="ps_pv", bufs=2, space="PSUM") as ps_pv,
            tc.tile_pool(name="ps_dn", bufs=1, space="PSUM") as ps_dn,
            tc.tile_pool(name="ps_inj", bufs=1, space="PSUM") as ps_inj,
        ):
            # ---- DMA issue order: wq+bq, xq, wk+bk, xk, premasks, wv+bv,
            # xv, wp.  Completion roughly follows issue order, so the PE
            # opening (warmup -> q0 -> q1 -> k0) is fed just in time.
            wq_sb = const.tile([P, KSUB, HPC * DOUT], BF)
            nc.sync.dma_start(wq_sb[:], wq_d.rearrange("(o p) m -> p o m", p=P))
            bqp_sb = const.tile([P, NPAIR], F32)
            nc.sync.dma_start(bqp_sb[:], bqp_d[:])
            xq_sb = xin.tile([P, KSUB, SEQ], BF)
            for o in range(KSUB):
                nc.sync.dma_start(
                    xq_sb[:, o, :], xq_d.rearrange("(o p) s -> p o s", p=P)[:, o, :]
                )
            wk_sb = const.tile([P, KSUB, HPC * DOUT], BF)
            nc.sync.dma_start(wk_sb[:], wk_d.rearrange("(o p) m -> p o m", p=P))
            bkp_sb = const.tile([P, NPAIR], F32)
            nc.sync.dma_start(bkp_sb[:], bkp_d[:])
            xk_sb = xin.tile([P, KSUB, SEQ], BF)
            for o in range(KSUB):
                nc.sync.dma_start(
                    xk_sb[:, o, :], xk_d.rearrange("(o p) s -> p o s", p=P)[:, o, :]
                )
            # prefetch the first 3 mask tiles of block (0,0) ahead of xv so
            # the first DVE mask-mults are not gated on the 4MB xv stream
            premask = []
            for kt in range(3):
                mt = maskp.tile([P, 1024], BF, tag="mt", name="mt")
                nc.sync.dma_start(mt[:], mk_d[kt * P : (kt + 1) * P, 0:1024])
                premask.append(mt)
            wv_sb = const.tile([P, KSUB, HPC * DOUT], BF)
            nc.sync.dma_start(wv_sb[:], wv_d.rearrange("(o p) m -> p o m", p=P))
            bvr_sb = const.tile([P, HPC * DOUT], F32)
            nc.sync.dma_start(bvr_sb[:], bvr_d[:])
            xv_sb = xin.tile([P, KSUB, SEQ], BF)
            for o in range(KSUB):
                nc.sync.dma_start(
                    xv_sb[:, o, :], xv_d.rearrange("(o p) s -> p o s", p=P)[:, o, :]
                )
            wp_sb = const.tile([P, NPAIR, DIN], BF)
            nc.sync.dma_start(wp_sb[:], wp_d.rearrange("(o p) n -> p o n", p=P))

            # ---- small constants -------------------------------------------
            ones_sb = const.tile([P, 1], BF)
            nc.vector.memset(ones_sb[:], 1.0)
            ones64_bf = const.tile([1, DOUT], BF)
            nc.vector.memset(ones64_bf[:], 1.0)
            warm_rhs = const.tile([P, 512], BF)
            nc.vector.memset(warm_rhs[:], 1.0)

            def warm_inj(n):
                wps = ps_inj.tile([P, 512], F32, tag="inj", name="wps", bufs=1)
                for i in range(n):
                    nc.tensor.matmul(
                        wps[:, 0:256],
                        warm_rhs[:, 0:P],
                        warm_rhs[:, 0:256],
                        start=True,
                        stop=True,
                        skip_group_check=True,
                    )

            def warm_fill(n):
                warm_ps = ps_s.tile(
                    [P, 1024], F32, tag="s0", name="warm_ps", bufs=1
                )
                for i in range(n):
                    nc.tensor.matmul(
                        warm_ps[:, 0:256],
                        warm_rhs[:, 0:P],
                        warm_rhs[:, 0:256],
                        start=True,
                        stop=True,
                        skip_group_check=True,
                    )

            # ---- persistent intermediates ----------------------------------
            qhT = proj.tile([P, NPAIR, SEQ], BF)  # [2-head dout, pair, seq]
            khT = proj.tile([P, NPAIR, SEQ], BF)
            vsb = proj.tile([P, NKT, HPC * DOUT], BF)  # v natural [seq, dout]
            ynT = proj.tile([P, NPAIR, SEQ], BF)  # normalized y.T

            vsb4 = vsb.rearrange("p k (h c) -> p k h c", c=DOUT)
            bvr4 = bvr_sb.rearrange("p (h c) -> p h c", c=DOUT)

            # ---- projection emitters ---------------------------------------
            # full 512-col q/k tiles for the opening phase
            def proj_qk_full(w_sb, x_sb, b_sb, o_sb, pair, qts=None):
                for qt in qts if qts is not None else range(SEQ // 512):
                    pps = ps_s.tile(
                        [P, 1024], F32, tag=f"s{qt % 2}", name="pps", bufs=1
                    )
                    for o in range(KSUB):
                        nc.tensor.matmul(
                            pps[:, :512],
                            w_sb[:, o, pair * P : (pair + 1) * P],
                            x_sb[:, o, qt * 512 : (qt + 1) * 512],
                            start=(o == 0),
                            stop=(o == KSUB - 1),
                        )
                    nc.vector.tensor_tensor(
                        o_sb[:, pair, qt * 512 : (qt + 1) * 512],
                        pps[:, :512],
                        b_sb[:, pair : pair + 1].to_broadcast((P, 512)),
                        ADD,
                    )

            # 256-col q/k unit for injection into attention kt slots
            def qk_unit(w_sb, x_sb, b_sb, o_sb, pair, u):
                def emit():
                    pps = ps_inj.tile(
                        [P, 512], F32, tag="inj", name="ppsu", bufs=1
                    )
                    c0 = u * 256
                    for o in range(KSUB):
                        nc.tensor.matmul(
                            pps[:, :256],
                            w_sb[:, o, pair * P : (pair + 1) * P],
                            x_sb[:, o, c0 : c0 + 256],
                            start=(o == 0),
                            stop=(o == KSUB - 1),
                        )
                    nc.vector.tensor_tensor(
                        o_sb[:, pair, c0 : c0 + 256],
                        pps[:, :256],
                        b_sb[:, pair : pair + 1].to_broadcast((P, 256)),
                        ADD,
                    )

                return emit

            # single-pair v unit covering TWO st-tiles (one psum alloc)
            def v_unit2(pair, st):
                def emit():
                    vps = ps_inj.tile(
                        [P, 512], F32, tag="inj", name="vps", bufs=1
                    )
                    for k in range(2):
                        for o in range(KSUB):
                            nc.tensor.matmul(
                                vps[:, k * P : (k + 1) * P],
                                xv_sb[:, o, (st + k) * P : (st + k + 1) * P],
                                wv_sb[:, o, pair * P : (pair + 1) * P],
                                start=(o == 0),
                                stop=(o == KSUB - 1),
                            )
                    for k in range(2):
                        nc.vector.tensor_tensor(
                            vsb4[:, st + k, pair * 2 : pair * 2 + 2, :],
                            vps[:, k * P : (k + 1) * P].rearrange(
                                "p (h c) -> p h c", c=DOUT
                            ),
                            bvr4[:, pair * 2 : pair * 2 + 2, :],
                            ADD,
                        )

                return emit

            # output-projection unit: one (st, ntile) [128, 512] tile of C
            def c_unit(st, ntile, ost_eng="dve", tag="inj"):
                def emit():
                    if tag == "inj":
                        cps = ps_inj.tile(
                            [P, 512], F32, tag="inj", name="cps", bufs=1
                        )
                    else:
                        cps = ps_s.tile(
                            [P, 1024], F32, tag=tag, name="cps", bufs=1
                        )[:, :512]
                    for pair in range(NPAIR):
                        nc.tensor.matmul(
                            cps[:],
                            ynT[:, pair, st * P : (st + 1) * P],
                            wp_sb[:, pair, ntile * 512 : (ntile + 1) * 512],
                            start=(pair == 0),
                            stop=(pair == NPAIR - 1),
                        )
                    ost = epi.tile([P, 512], BF, tag="ost", name="ost", bufs=3)
                    if ost_eng == "dve":
                        nc.vector.tensor_copy(ost[:], cps[:])
                    else:
                        nc.scalar.copy(ost[:], cps[:])
                    nc.sync.dma_start(
                        out_d[
                            st * P : (st + 1) * P,
                            ntile * 512 : (ntile + 1) * 512,
                        ],
                        ost[:],
                    )

                return emit

            # ---- epilogue --------------------------------------------------
            # stage 1 (den_copy): copy the 4 den rows of a finished block out
            # of the dn bank (frees it for reallocation).
            # stage 2 (normalize): per qi half: 1/den, bf16 cast, col-packed
            # K=1 broadcast into rdb, one full-height multiply -> ynT.
            pending = []  # (pair, qh, qi, dens, pv_sbs)

            def den_copy(dn_ps, pv_sbs, pair, qh):
                for qi in range(2):
                    dens = []
                    for h2 in range(2):
                        row = 32 * qi + 64 * h2
                        den = epi.tile(
                            [1, 512], F32, tag=f"den{qi}{h2}", name="den", bufs=1
                        )
                        nc.vector.tensor_copy(den[:], dn_ps[row : row + 1, :])
                        dens.append(den)
                    pending.append((pair, qh, qi, dens, pv_sbs))

            def normalize_piece():
                pair_, qh_, qi, dens, pv_sbs = pending.pop(0)
                q0 = (qh_ * 2 + qi) * 512
                rcpbs = []
                for h2 in range(2):
                    rcp = epi.tile(
                        [1, 512], F32, tag=f"rcp{h2}", name="rcp", bufs=1
                    )
                    nc.vector.reciprocal_approx_fast(rcp[:], dens[h2][:])
                    rcpb = epi.tile(
                        [1, 512], BF, tag=f"rcpb{h2}", name="rcpb", bufs=1
                    )
                    nc.vector.tensor_copy(rcpb[:], rcp[:])
                    rcpbs.append(rcpb)
                rdb = ps_s.tile([P, 1024], F32, tag="s1", name="rdb", bufs=1)
                for h2 in range(2):
                    nc.tensor.matmul(
                        rdb[h2 * DOUT : (h2 + 1) * DOUT, :512],
                        ones64_bf[:],
                        rcpbs[h2][:],
                        start=True,
                        stop=True,
                        tile_position=(0, h2 * DOUT),
                        skip_group_check=True,
                    )
                nc.vector.tensor_mul(
                    ynT[:, pair_, q0 : q0 + 512],
                    pv_sbs[qi][:],
                    rdb[:, :512],
                )

            # ---- attention block -------------------------------------------
            # carry: closures from the previous block, keyed by kt slot
            def attn_block(pair, qh, inj, carry, use_premask=False, pv_lag=0):
                first = not carry
                pvs = []
                for i in range(2):
                    pv = ps_pv.tile([P, 512], F32, tag="pv", name=f"pv{i}")
                    nc.vector.memset(pv[:], 0.0)
                    pvs.append(pv)
                dn_box = {}
                if first:
                    dn_box["dn"] = ps_dn.tile([P, 512], F32, tag="dn", name="dn")
                    nc.vector.memset(dn_box["dn"][:], 0.0)

                ptm_ring = {}

                def dn_mms(kt):
                    ptms = ptm_ring[kt]
                    for qi in range(2):
                        for h2 in range(2):
                            row = 32 * qi + 64 * h2
                            nc.tensor.matmul(
                                dn_box["dn"][row : row + 1, :],
                                ones_sb[:],
                                ptms[h2][:, qi * 512 : (qi + 1) * 512],
                                start=False,
                                stop=(kt == NKT - 1),
                                tile_position=(0, row),
                                skip_group_check=True,
                            )

                def pv_mms(kt):
                    ptms = ptm_ring[kt]
                    for qi in range(2):
                        for h2 in range(2):
                            nc.tensor.matmul(
                                pvs[qi][h2 * DOUT : (h2 + 1) * DOUT, :],
                                vsb4[:, kt, pair * 2 + h2, :],
                                ptms[h2][:, qi * 512 : (qi + 1) * 512],
                                start=False,
                                stop=(kt == NKT - 1),
                                tile_position=(0, h2 * DOUT),
                                skip_group_check=True,
                            )

                # dn bank of the PREVIOUS block frees after the kt-2 den
                # copies; this block's dn bank is allocated at kt 4 and the
                # denominator matmuls catch up 3 kt-groups per slot.
                def dn_target(kt):
                    if first:
                        return max(0, kt - 2)
                    if kt < 5:
                        return 0
                    return min(kt - 2, 3 * (kt - 4))

                # PV issue schedule: lag >=3; for blocks whose v-projection
                # is injected just-in-time, also lag behind the v-units
                # (2 st-tiles per unit, 1 unit per kt from kt 1 / kt 6).
                def pv_target(kt):
                    if kt < 3:
                        return 0
                    t = (kt - 2) * NKT // 13
                    if pv_lag > 0:
                        t = min(t, max(0, 2 * (kt - pv_lag)))
                    elif pv_lag < 0:
                        t = min(t, kt - 2)
                    return min(t, kt - 1, NKT)

                pv_done = [0]
                dn_done = [0]

                for kt in range(NKT):
                    if use_premask and kt < len(premask):
                        mt = premask[kt]
                    else:
                        mt = maskp.tile([P, 1024], BF, tag="mt", name="mt")
                        nc.sync.dma_start(
                            mt[:],
                            mk_d[kt * P : (kt + 1) * P, qh * 1024 : (qh + 1) * 1024],
                        )
                    ptms = []

                    def s_head(h2):
                        hs = slice(h2 * DOUT, (h2 + 1) * DOUT)
                        sps = ps_s.tile(
                            [P, 1024], F32, tag=f"s{h2}", name="sps", bufs=1
                        )
                        for qi in range(2):
                            q0 = (qh * 2 + qi) * 512
                            nc.tensor.matmul(
                                sps[:, qi * 512 : (qi + 1) * 512],
                                khT[hs, pair, kt * P : (kt + 1) * P],
                                qhT[hs, pair, q0 : q0 + 512],
                                start=True,
                                stop=True,
                            )
                        pt = ptp.tile([P, 1024], BF, tag=f"pt{h2}", name="pt", bufs=2)
                        nc.scalar.activation(pt[:], sps[:], EXP, scale=0.125)
                        ptm = ptp.tile(
                            [P, 1024], BF, tag=f"ptm{h2}", name="ptm", bufs=6
                        )
                        nc.vector.tensor_mul(ptm[:], pt[:], mt[:])
                        ptms.append(ptm)

                    # the dn matmuls for kt-1 are emitted BETWEEN the two S
                    # pairs: they are ready work that absorbs the semaphore
                    # wait of S(h1) on exp(h1, kt-1)
                    s_head(0)
                    if not first and kt == 4:
                        dn_box["dn"] = ps_dn.tile(
                            [P, 512], F32, tag="dn", name="dn"
                        )
                        nc.vector.memset(dn_box["dn"][:], 0.0)
                    while dn_done[0] < dn_target(kt):
                        dn_mms(dn_done[0])
                        dn_done[0] += 1
                    s_head(1)
                    ptm_ring[kt] = ptms
                    # cross-block carry / spread epilogue of previous block
                    if kt < len(carry):
                        for fn in carry[kt]:
                            fn()
                    # PV matmuls: issue up to the static target
                    while pv_done[0] < pv_target(kt):
                        pv_mms(pv_done[0])
                        pv_done[0] += 1
                    # injected work for this slot
                    for fn in inj.get(kt, ()):
                        fn()

                # carry out: finish dn/PV + drains + epilogue inside the
                # next block's early kt slots
                pv_sbs = []

                def carry0():
                    while dn_done[0] < NKT:
                        dn_mms(dn_done[0])
                        dn_done[0] += 1
                    while pv_done[0] < NKT - 1:
                        pv_mms(pv_done[0])
                        pv_done[0] += 1

                def carry1():
                    pv_mms(NKT - 1)
                    pv_done[0] += 1
                    for qi in range(2):
                        pv_sb = epi.tile(
                            [P, 512], F32, tag="pvsb", name="pv_sb", bufs=3
                        )
                        nc.vector.tensor_copy(pv_sb[:], pvs[qi][:])
                        pv_sbs.append(pv_sb)

                def carry2():
                    den_copy(dn_box["dn"], pv_sbs, pair, qh)

                return [[carry0], [carry1], [carry2], [normalize_piece],
                        [], [normalize_piece]]

            # ---- opening ---------------------------------------------------
            warm_fill(N_WARM)
            proj_qk_full(wq_sb, xq_sb, bqp_sb, qhT, 0)
            proj_qk_full(wq_sb, xq_sb, bqp_sb, qhT, 1)
            proj_qk_full(wk_sb, xk_sb, bkp_sb, khT, 0, qts=[0])

            # ---- attention blocks with injections --------------------------
            # (0,0): one injection unit per kt -- v-proj pair0 on odd kts,
            # the remaining 3/4 of the pair-0 k projection on even kts
            # (unit u covers score kt tiles 2u..2u+1, first read at kt=2u)
            inj00 = {2 * i + 1: [v_unit2(0, 2 * i)] for i in range(8)}
            for j, u in enumerate(range(2, 8)):
                inj00.setdefault(2 * j, []).append(
                    qk_unit(wk_sb, xk_sb, bkp_sb, khT, 0, u)
                )
            carry = attn_block(0, 0, inj00, [], use_premask=True, pv_lag=-1)
            # (0,1): k-proj pair1 (256-col units), 1 per kt from kt=6
            inj01 = {
                6 + u: [qk_unit(wk_sb, xk_sb, bkp_sb, khT, 1, u)]
                for u in range(8)
            }
            carry = attn_block(0, 1, inj01, carry)
            # (1,0): v-proj pair1
            inj10 = {2 + i: [v_unit2(1, 2 * i)] for i in range(8)}
            carry = attn_block(1, 0, inj10, carry, pv_lag=4)
            # (1,1): output projection for q 0..1024 (st 0..7); st 0..3
            # unlock after the kt-3 normalize, st 4..7 after kt-5
            inj11 = {}
            units = [c_unit(st, nt) for st in range(8) for nt in range(2)]
            slots = [4, 5, 6, 6, 7, 7, 8, 8, 9, 9, 10, 10, 11, 12, 13, 14]
            for u, sl in zip(units, slots):
                inj11.setdefault(sl, []).append(u)
            carry = attn_block(1, 1, inj11, carry)

            # ---- tail ------------------------------------------------------
            for fns in carry[:3]:
                for fn in fns:
                    fn()
            normalize_piece()
            for i, st in enumerate(range(8, 12)):
                c_unit(st, 0, "act", tag=["inj", "s0"][i % 2])()
                c_unit(st, 1, "dve", tag="s1" if i % 2 else "inj")()
            normalize_piece()
            for i, st in enumerate(range(12, 16)):
                c_unit(st, 0, "act", tag=["inj", "s0"][i % 2])()
                c_unit(st, 1, "dve", tag="s1" if i % 2 else "inj")()

    nc.finalize()
    return nc


def make_in_maps(query, key, value, mask, Wq, bq, Wk, bk, Wv, bv, Wp, bp):
    """Shard + pre-layout the full inputs into 8 per-core input dicts."""
    in_maps = []
    for c in range(8):
        n = c // 4
        h0 = HPC * (c % 4)
        hs = slice(h0, h0 + HPC)

        def t_bf(x):  # [SEQ, DIN] -> contiguous [DIN, SEQ] bf16
            return np.ascontiguousarray(x.T).astype(BF_NP)

        # (H', DIN, DOUT) -> (DIN, H'*DOUT), head-major columns
        def w_bf(W):
            return np.ascontiguousarray(
                W[hs].transpose(1, 0, 2).reshape(DIN, HPC * DOUT)
            ).astype(BF_NP)

        # per-pair per-partition bias: [128, 2], col p = concat of heads (2p, 2p+1)
        def b_pair(b):
            return np.ascontiguousarray(b[hs].reshape(NPAIR, P).T).astype(np.float32)

        in_maps.append(
            {
                "xqT": t_bf(query[n]),
                "xkT": t_bf(key[n]),
                "xvT": t_bf(value[n]),
                "maskT": np.ascontiguousarray((~mask[n]).T).astype(BF_NP),
                "wq": w_bf(Wq),
                "wk": w_bf(Wk),
                "wv": w_bf(Wv),
                "wp": np.ascontiguousarray(
                    Wp[h0 * DOUT : (h0 + HPC) * DOUT, :]
                ).astype(BF_NP),
                "bqp": b_pair(bq),
                "bkp": b_pair(bk),
                "bvr": np.ascontiguousarray(
                    np.tile(bv[hs].reshape(1, HPC * DOUT), (P, 1))
                ).astype(np.float32),
            }
        )
    return in_maps


def kernel(**inputs):
    global _NC_CACHE
    from concourse.bass_utils import run_bass_kernel_spmd

    if _NC_CACHE is None:
        _NC_CACHE = build_bass()
    nc = _NC_CACHE

    in_maps = make_in_maps(**inputs)
    res = run_bass_kernel_spmd(nc, in_maps, core_ids=list(range(8))).results
    parts = [res[c]["out"].astype(np.float32) for c in range(8)]
    bp = inputs["bp"]
    out = np.stack(
        [
            parts[0] + parts[1] + parts[2] + parts[3] + bp[None, :],
            parts[4] + parts[5] + parts[6] + parts[7] + bp[None, :],
        ]
    )
    return out.astype(np.float32)


# revision 47
# speedup vs baseline: 1.0519x; 1.0519x over previous
"""Multi-head attention (N=2, K=2048, DIN=1024, H=16, DOUT=64) on 8 TRN2 NeuronCores.

Sharding: data-parallel over batch N (cores 0-3 -> n=0, cores 4-7 -> n=1),
tensor-parallel over heads (4 heads per core). Each core computes its 4 heads'
attention plus the partial output projection over its head-feature slice of Wp;
the host sums the 4 partials per batch element and adds the output bias.

Schedule (final): the attention phase targets joint ScalarE(exp)/DVE/PE
saturation at ~2.4us per kt tile.  All non-attention PE work (pair-1 k
projection, both v projections, output projection) is injected into the
attention kt loops through a DEDICATED psum bank so it never perturbs the
S-tile rotation; each head's S psum is its own single-buffered 2-bank tag so
S(h,kt+1) waits only on exp(h,kt):

  opening: warmup burst (HAM ramp, overlaps xq DMA), proj q pair0 + pair1
           (xq only), k pair0 (xk), then attention starts.
  block (0,0): + v-proj pair0 (2 st-tiles/unit) just-in-time for lagged PV
  block (0,1): + k-proj pair1
  block (1,0): + v-proj pair1
  block (1,1): + output projection for q 0..1024
  tail:    last epilogues + output projection for q 1024..2048

PSUM (8 banks): S-h0 2, S-h1 2 (also rdb), pv accumulators 2, dn 1
(single-buffered: the previous block's den rows are DVE-copied out at kt 2
and the bank is re-allocated at kt 4), injection bank 1.

Per-core kernel structure (all matmuls bf16, fp32 PSUM accumulation):
  - host pre-transposes/casts activations to bf16 [DIN, SEQ].
  - q/k projected head-pair-wise to [dout(2 heads on partitions), seq];
    v projected per-pair to the natural [seq, dout] layout.
  - scores computed transposed (S.T = k_h @ q_h.T) so softmax probabilities
    are already in the layout the PV matmul needs; no max-subtraction
    (|S/8| < 10, exp safe in fp32); mask applied as 0/1 multiply AFTER exp.
  - PV matmuls lag the S/exp/mask stream by >=3 kt (ring of ptm tiles) and
    col-pack 2 heads per PSUM bank; softmax denominators accumulate via
    col-packed M=1 ones-matmuls (lagged) in the dn bank.
  - accumulators are DVE-zeroed and accumulating matmuls use start=False.
  - epilogue per qi half: 1/den via custom-DVE reciprocal on base-0 copies,
    bf16 cast, two col-packed K=1 outer products broadcast 1/den across both
    heads' 64 rows, one full-height multiply normalizes 128 x 512 at once.
  - epilogue work is spread over the NEXT block's kt slots (den copies kt 2,
    normalize kts 3/5) and the final dn/PV matmuls + pv drains are emitted
    inside the next block's first two kt slots (cross-block carry).
  - output tiles drain on DVE inside block (1,1), on ScalarE in the tail;
    output is written bf16 (halves the out-DMA).
"""

import numpy as np
import ml_dtypes

import concourse.bass as bass
import concourse.mybir as mybir
from concourse import bacc
from concourse.tile import TileContext

P = 128
SEQ = 2048
DIN = 1024
DOUT = 64
H = 16
N = 2
HPC = 4  # heads per core
NPAIR = 2  # head pairs per core
KSUB = DIN // P  # 8 contraction subtiles for projections
NKT = SEQ // P  # 16 seq_k tiles of 128
BF = mybir.dt.bfloat16
F32 = mybir.dt.float32
BF_NP = ml_dtypes.bfloat16

N_WARM = 88  # opening warmup matmuls (dependency-free, ramps HAM to 8/8)

_NC_CACHE = None


def build_bass():
    nc = bacc.Bacc()

    xq_d = nc.declare_dram_parameter("xqT", [DIN, SEQ], BF, isOutput=False)
    xk_d = nc.declare_dram_parameter("xkT", [DIN, SEQ], BF, isOutput=False)
    xv_d = nc.declare_dram_parameter("xvT", [DIN, SEQ], BF, isOutput=False)
    mk_d = nc.declare_dram_parameter("maskT", [SEQ, SEQ], BF, isOutput=False)
    wq_d = nc.declare_dram_parameter("wq", [DIN, HPC * DOUT], BF, isOutput=False)
    wk_d = nc.declare_dram_parameter("wk", [DIN, HPC * DOUT], BF, isOutput=False)
    wv_d = nc.declare_dram_parameter("wv", [DIN, HPC * DOUT], BF, isOutput=False)
    wp_d = nc.declare_dram_parameter("wp", [HPC * DOUT, DIN], BF, isOutput=False)
    bqp_d = nc.declare_dram_parameter("bqp", [P, NPAIR], F32, isOutput=False)
    bkp_d = nc.declare_dram_parameter("bkp", [P, NPAIR], F32, isOutput=False)
    bvr_d = nc.declare_dram_parameter("bvr", [P, HPC * DOUT], F32, isOutput=False)
    out_d = nc.declare_dram_parameter("out", [SEQ, DIN], BF, isOutput=True)

    ADD = mybir.AluOpType.add
    EXP = mybir.ActivationFunctionType.Exp

    with TileContext(nc) as tc:
        with (
            tc.tile_pool(name="const", bufs=1) as const,
            tc.tile_pool(name="xin", bufs=1) as xin,
            tc.tile_pool(name="proj", bufs=1) as proj,
            tc.tile_pool(name="maskp", bufs=3) as maskp,
            tc.tile_pool(name="ptp", bufs=2) as ptp,
            tc.tile_pool(name="epi", bufs=2) as epi,
            tc.tile_pool(name="ps_s", bufs=1, space="PSUM") as ps_s,
            tc.tile_pool(name="ps_pv", bufs=2, space="PSUM") as ps_pv,
            tc.tile_pool(name="ps_dn", bufs=1, space="PSUM") as ps_dn,
            tc.tile_pool(name="ps_inj", bufs=1, space="PSUM") as ps_inj,
        ):
            # ---- DMA issue order: wq+bq, xq, wk+bk, xk, premasks, wv+bv,
            # xv, wp.  Completion roughly follows issue order, so the PE
            # opening (warmup -> q0 -> q1 -> k0) is fed just in time.
            wq_sb = const.tile([P, KSUB, HPC * DOUT], BF)
            nc.sync.dma_start(wq_sb[:], wq_d.rearrange("(o p) m -> p o m", p=P))
            bqp_sb = const.tile([P, NPAIR], F32)
            nc.sync.dma_start(bqp_sb[:], bqp_d[:])
            xq_sb = xin.tile([P, KSUB, SEQ], BF)
            for o in range(KSUB):
                nc.sync.dma_start(
                    xq_sb[:, o, :], xq_d.rearrange("(o p) s -> p o s", p=P)[:, o, :]
                )
            wk_sb = const.tile([P, KSUB, HPC * DOUT], BF)
            nc.sync.dma_start(wk_sb[:], wk_d.rearrange("(o p) m -> p o m", p=P))
            bkp_sb = const.tile([P, NPAIR], F32)
            nc.sync.dma_start(bkp_sb[:], bkp_d[:])
            xk_sb = xin.tile([P, KSUB, SEQ], BF)
            for o in range(KSUB):
                nc.sync.dma_start(
                    xk_sb[:, o, :], xk_d.rearrange("(o p) s -> p o s", p=P)[:, o, :]
                )
            # prefetch the first 3 mask tiles of block (0,0) ahead of xv so
            # the first DVE mask-mults are not gated on the 4MB xv stream
            premask = []
            for kt in range(3):
                mt = maskp.tile([P, 1024], BF, tag="mt", name="mt")
                nc.sync.dma_start(mt[:], mk_d[kt * P : (kt + 1) * P, 0:1024])
                premask.append(mt)
            wv_sb = const.tile([P, KSUB, HPC * DOUT], BF)
            nc.sync.dma_start(wv_sb[:], wv_d.rearrange("(o p) m -> p o m", p=P))
            bvr_sb = const.tile([P, HPC * DOUT], F32)
            nc.sync.dma_start(bvr_sb[:], bvr_d[:])
            xv_sb = xin.tile([P, KSUB, SEQ], BF)
            for o in range(KSUB):
                nc.sync.dma_start(
                    xv_sb[:, o, :], xv_d.rearrange("(o p) s -> p o s", p=P)[:, o, :]
                )
            wp_sb = const.tile([P, NPAIR, DIN], BF)
            nc.sync.dma_start(wp_sb[:], wp_d.rearrange("(o p) n -> p o n", p=P))

            # ---- small constants -------------------------------------------
            ones_sb = const.tile([P, 1], BF)
            nc.vector.memset(ones_sb[:], 1.0)
            ones64_bf = const.tile([1, DOUT], BF)
            nc.vector.memset(ones64_bf[:], 1.0)
            warm_rhs = const.tile([P, 512], BF)
            nc.vector.memset(warm_rhs[:], 1.0)

            def warm_inj(n):
                wps = ps_inj.tile([P, 512], F32, tag="inj", name="wps", bufs=1)
                for i in range(n):
                    nc.tensor.matmul(
                        wps[:, 0:256],
                        warm_rhs[:, 0:P],
                        warm_rhs[:, 0:256],
                        start=True,
                        stop=True,
                        skip_group_check=True,
                    )

            def warm_fill(n):
                warm_ps = ps_s.tile(
                    [P, 1024], F32, tag="s0", name="warm_ps", bufs=1
                )
                for i in range(n):
                    nc.tensor.matmul(
                        warm_ps[:, 0:256],
                        warm_rhs[:, 0:P],
                        warm_rhs[:, 0:256],
                        start=True,
                        stop=True,
                        skip_group_check=True,
                    )

            # ---- persistent intermediates ----------------------------------
            qhT = proj.tile([P, NPAIR, SEQ], BF)  # [2-head dout, pair, seq]
            khT = proj.tile([P, NPAIR, SEQ], BF)
            vsb = proj.tile([P, NKT, HPC * DOUT], BF)  # v natural [seq, dout]
            ynT = proj.tile([P, NPAIR, SEQ], BF)  # normalized y.T

            vsb4 = vsb.rearrange("p k (h c) -> p k h c", c=DOUT)
            bvr4 = bvr_sb.rearrange("p (h c) -> p h c", c=DOUT)

            # ---- projection emitters ---------------------------------------
            # full 512-col q/k tiles for the opening phase
            def proj_qk_full(w_sb, x_sb, b_sb, o_sb, pair, qts=None):
                for qt in qts if qts is not None else range(SEQ // 512):
                    pps = ps_s.tile(
                        [P, 1024], F32, tag=f"s{qt % 2}", name="pps", bufs=1
                    )
                    for o in range(KSUB):
                        nc.tensor.matmul(
                            pps[:, :512],
                            w_sb[:, o, pair * P : (pair + 1) * P],
                            x_sb[:, o, qt * 512 : (qt + 1) * 512],
                            start=(o == 0),
                            stop=(o == KSUB - 1),
                        )
                    nc.vector.tensor_tensor(
                        o_sb[:, pair, qt * 512 : (qt + 1) * 512],
                        pps[:, :512],
                        b_sb[:, pair : pair + 1].to_broadcast((P, 512)),
                        ADD,
                    )

            # 256-col q/k unit for injection into attention kt slots
            def qk_unit(w_sb, x_sb, b_sb, o_sb, pair, u):
                def emit():
                    pps = ps_inj.tile(
                        [P, 512], F32, tag="inj", name="ppsu", bufs=1
                    )
                    c0 = u * 256
                    for o in range(KSUB):
                        nc.tensor.matmul(
                            pps[:, :256],
                            w_sb[:, o, pair * P : (pair + 1) * P],
                            x_sb[:, o, c0 : c0 + 256],
                            start=(o == 0),
                            stop=(o == KSUB - 1),
                        )
                    nc.vector.tensor_tensor(
                        o_sb[:, pair, c0 : c0 + 256],
                        pps[:, :256],
                        b_sb[:, pair : pair + 1].to_broadcast((P, 256)),
                        ADD,
                    )

                return emit

            # single-pair v unit covering TWO st-tiles (one psum alloc)
            def v_unit2(pair, st):
                def emit():
                    vps = ps_inj.tile(
                        [P, 512], F32, tag="inj", name="vps", bufs=1
                    )
                    for k in range(2):
                        for o in range(KSUB):
                            nc.tensor.matmul(
                                vps[:, k * P : (k + 1) * P],
                                xv_sb[:, o, (st + k) * P : (st + k + 1) * P],
                                wv_sb[:, o, pair * P : (pair + 1) * P],
                                start=(o == 0),
                                stop=(o == KSUB - 1),
                            )
                    for k in range(2):
                        nc.vector.tensor_tensor(
                            vsb4[:, st + k, pair * 2 : pair * 2 + 2, :],
                            vps[:, k * P : (k + 1) * P].rearrange(
                                "p (h c) -> p h c", c=DOUT
                            ),
                            bvr4[:, pair * 2 : pair * 2 + 2, :],
                            ADD,
                        )

                return emit

            # output-projection unit: one (st, ntile) [128, 512] tile of C
            def c_unit(st, ntile, ost_eng="dve", tag="inj"):
                def emit():
                    if tag == "inj":
                        cps = ps_inj.tile(
                            [P, 512], F32, tag="inj", name="cps", bufs=1
                        )
                    else:
                        cps = ps_s.tile(
                            [P, 1024], F32, tag=tag, name="cps", bufs=1
                        )[:, :512]
                    for pair in range(NPAIR):
                        nc.tensor.matmul(
                            cps[:],
                            ynT[:, pair, st * P : (st + 1) * P],
                            wp_sb[:, pair, ntile * 512 : (ntile + 1) * 512],
                            start=(pair == 0),
                            stop=(pair == NPAIR - 1),
                        )
                    ost = epi.tile([P, 512], BF, tag="ost", name="ost", bufs=3)
                    if ost_eng == "dve":
                        nc.vector.tensor_copy(ost[:], cps[:])
                    else:
                        nc.scalar.copy(ost[:], cps[:])
                    nc.sync.dma_start(
                        out_d[
                            st * P : (st + 1) * P,
                            ntile * 512 : (ntile + 1) * 512,
                        ],
                        ost[:],
                    )

                return emit

            # ---- epilogue --------------------------------------------------
            # stage 1 (den_copy): copy the 4 den rows of a finished block out
            # of the dn bank (frees it for reallocation).
            # stage 2 (normalize): per qi half: 1/den, bf16 cast, col-packed
            # K=1 broadcast into rdb, one full-height multiply -> ynT.
            pending = []  # (pair, qh, qi, dens, pv_sbs)

            def den_copy(dn_ps, pv_sbs, pair, qh):
                for qi in range(2):
                    dens = []
                    for h2 in range(2):
                        row = 32 * qi + 64 * h2
                        den = epi.tile(
                            [1, 512], F32, tag=f"den{qi}{h2}", name="den", bufs=1
                        )
                        nc.vector.tensor_copy(den[:], dn_ps[row : row + 1, :])
                        dens.append(den)
                    pending.append((pair, qh, qi, dens, pv_sbs))

            def normalize_piece():
                pair_, qh_, qi, dens, pv_sbs = pending.pop(0)
                q0 = (qh_ * 2 + qi) * 512
                rcpbs = []
                for h2 in range(2):
                    rcp = epi.tile(
                        [1, 512], F32, tag=f"rcp{h2}", name="rcp", bufs=1
                    )
                    nc.vector.reciprocal_approx_fast(rcp[:], dens[h2][:])
                    rcpb = epi.tile(
                        [1, 512], BF, tag=f"rcpb{h2}", name="rcpb", bufs=1
                    )
                    nc.vector.tensor_copy(rcpb[:], rcp[:])
                    rcpbs.append(rcpb)
                rdb = ps_s.tile([P, 1024], F32, tag="s1", name="rdb", bufs=1)
                for h2 in range(2):
                    nc.tensor.matmul(
                        rdb[h2 * DOUT : (h2 + 1) * DOUT, :512],
                        ones64_bf[:],
                        rcpbs[h2][:],
                        start=True,
                        stop=True,
                        tile_position=(0, h2 * DOUT),
                        skip_group_check=True,
                    )
                nc.vector.tensor_mul(
                    ynT[:, pair_, q0 : q0 + 512],
                    pv_sbs[qi][:],
                    rdb[:, :512],
                )

            # ---- attention block -------------------------------------------
            # carry: closures from the previous block, keyed by kt slot
            def attn_block(pair, qh, inj, carry, use_premask=False, pv_lag=0):
                first = not carry
                pvs = []
                for i in range(2):
                    pv = ps_pv.tile([P, 512], F32, tag="pv", name=f"pv{i}")
                    nc.vector.memset(pv[:], 0.0)
                    pvs.append(pv)
                dn_box = {}
                if first:
                    dn_box["dn"] = ps_dn.tile([P, 512], F32, tag="dn", name="dn")
                    nc.vector.memset(dn_box["dn"][:], 0.0)

                ptm_ring = {}

                def dn_mms(kt):
                    ptms = ptm_ring[kt]
                    for qi in range(2):
                        for h2 in range(2):
                            row = 32 * qi + 64 * h2
                            nc.tensor.matmul(
                                dn_box["dn"][row : row + 1, :],
                                ones_sb[:],
                                ptms[h2][:, qi * 512 : (qi + 1) * 512],
                                start=False,
                                stop=(kt == NKT - 1),
                                tile_position=(0, row),
                                skip_group_check=True,
                            )

                def pv_mms(kt):
                    ptms = ptm_ring[kt]
                    for qi in range(2):
                        for h2 in range(2):
                            nc.tensor.matmul(
                                pvs[qi][h2 * DOUT : (h2 + 1) * DOUT, :],
                                vsb4[:, kt, pair * 2 + h2, :],
                                ptms[h2][:, qi * 512 : (qi + 1) * 512],
                                start=False,
                                stop=(kt == NKT - 1),
                                tile_position=(0, h2 * DOUT),
                                skip_group_check=True,
                            )

                # dn bank of the PREVIOUS block frees after the kt-2 den
                # copies; this block's dn bank is allocated at kt 4 and the
                # denominator matmuls catch up 3 kt-groups per slot.
                def dn_target(kt):
                    if first:
                        return max(0, kt - 1)
                    if kt < 5:
                        return 0
                    return min(kt - 1, 3 * (kt - 4))

                # PV issue schedule: lag >=3; for blocks whose v-projection
                # is injected just-in-time, also lag behind the v-units
                # (2 st-tiles per unit, 1 unit per kt from kt 1 / kt 6).
                def pv_target(kt):
                    if kt < 3:
                        return 0
                    t = (kt - 2) * NKT // 13
                    if pv_lag > 0:
                        t = min(t, max(0, 2 * (kt - pv_lag)))
                    elif pv_lag < 0:
                        t = min(t, kt - 2)
                    return min(t, kt - 1, NKT)

                pv_done = [0]
                dn_done = [0]

                for kt in range(NKT):
                    if use_premask and kt < len(premask):
                        mt = premask[kt]
                    else:
                        mt = maskp.tile([P, 1024], BF, tag="mt", name="mt")
                        nc.sync.dma_start(
                            mt[:],
                            mk_d[kt * P : (kt + 1) * P, qh * 1024 : (qh + 1) * 1024],
                        )
                    ptms = []

                    def s_head(h2):
                        hs = slice(h2 * DOUT, (h2 + 1) * DOUT)
                        sps = ps_s.tile(
                            [P, 1024], F32, tag=f"s{h2}", name="sps", bufs=1
                        )
                        for qi in range(2):
                            q0 = (qh * 2 + qi) * 512
                            nc.tensor.matmul(
                                sps[:, qi * 512 : (qi + 1) * 512],
                                khT[hs, pair, kt * P : (kt + 1) * P],
                                qhT[hs, pair, q0 : q0 + 512],
                                start=True,
                                stop=True,
                            )
                        pt = ptp.tile([P, 1024], BF, tag=f"pt{h2}", name="pt", bufs=2)
                        nc.scalar.activation(pt[:], sps[:], EXP, scale=0.125)
                        ptm = ptp.tile(
                            [P, 1024], BF, tag=f"ptm{h2}", name="ptm", bufs=6
                        )
                        nc.vector.tensor_mul(ptm[:], pt[:], mt[:])
                        ptms.append(ptm)

                    # the dn matmuls for kt-1 are emitted BETWEEN the two S
                    # pairs: they are ready work that absorbs the semaphore
                    # wait of S(h1) on exp(h1, kt-1)
                    s_head(0)
                    if not first and kt == 4:
                        dn_box["dn"] = ps_dn.tile(
                            [P, 512], F32, tag="dn", name="dn"
                        )
                        nc.vector.memset(dn_box["dn"][:], 0.0)
                    while dn_done[0] < dn_target(kt):
                        dn_mms(dn_done[0])
                        dn_done[0] += 1
                    s_head(1)
                    ptm_ring[kt] = ptms
                    # cross-block carry / spread epilogue of previous block
                    if kt < len(carry):
                        for fn in carry[kt]:
                            fn()
                    # PV matmuls: issue up to the static target
                    while pv_done[0] < pv_target(kt):
                        pv_mms(pv_done[0])
                        pv_done[0] += 1
                    # injected work for this slot
                    for fn in inj.get(kt, ()):
                        fn()

                # carry out: finish dn/PV + drains + epilogue inside the
                # next block's early kt slots
                pv_sbs = []

                def carry0():
                    while dn_done[0] < NKT:
                        dn_mms(dn_done[0])
                        dn_done[0] += 1
                    while pv_done[0] < NKT - 1:
                        pv_mms(pv_done[0])
                        pv_done[0] += 1

                def carry1():
                    pv_mms(NKT - 1)
                    pv_done[0] += 1
                    for qi in range(2):
                        pv_sb = epi.tile(
                            [P, 512], F32, tag="pvsb", name="pv_sb", bufs=3
                        )
                        nc.vector.tensor_copy(pv_sb[:], pvs[qi][:])
                        pv_sbs.append(pv_sb)

                def carry2():
                    den_copy(dn_box["dn"], pv_sbs, pair, qh)

                return [[carry0], [carry1], [carry2], [normalize_piece],
                        [], [normalize_piece]]

            # ---- opening ---------------------------------------------------
            warm_fill(N_WARM)
            proj_qk_full(wq_sb, xq_sb, bqp_sb, qhT, 0)
            proj_qk_full(wk_sb, xk_sb, bkp_sb, khT, 0, qts=[0])

            # ---- attention blocks with injections --------------------------
            # (0,0): one injection unit per kt -- v-proj pair0 on odd kts,
            # the remaining 3/4 of the pair-0 k projection on even kts
            # (unit u covers score kt tiles 2u..2u+1, first read at kt=2u)
            inj00 = {2 * i + 1: [v_unit2(0, 2 * i)] for i in range(8)}
            for j, u in enumerate(range(2, 8)):
                inj00.setdefault(2 * j, []).append(
                    qk_unit(wk_sb, xk_sb, bkp_sb, khT, 0, u)
                )
            carry = attn_block(0, 0, inj00, [], use_premask=True, pv_lag=-1)
            # (0,1): k-proj pair1 (256-col units) at kts 6-13, q-proj
            # pair1 at kts 0-5/14/15 -- one injection unit per kt slot
            inj01 = {
                6 + u: [qk_unit(wk_sb, xk_sb, bkp_sb, khT, 1, u)]
                for u in range(8)
            }
            for u, sl in zip(range(8), [0, 1, 2, 3, 4, 5, 14, 15]):
                inj01.setdefault(sl, []).append(
                    qk_unit(wq_sb, xq_sb, bqp_sb, qhT, 1, u)
                )
            carry = attn_block(0, 1, inj01, carry)
            # (1,0): v-proj pair1
            inj10 = {2 + i: [v_unit2(1, 2 * i)] for i in range(8)}
            carry = attn_block(1, 0, inj10, carry, pv_lag=4)
            # (1,1): output projection for q 0..1024 (st 0..7); st 0..3
            # unlock after the kt-3 normalize, st 4..7 after kt-5
            inj11 = {}
            units = [c_unit(st, nt) for st in range(8) for nt in range(2)]
            slots = [4, 5, 6, 6, 7, 7, 8, 8, 9, 9, 10, 10, 11, 12, 13, 14]
            for u, sl in zip(units, slots):
                inj11.setdefault(sl, []).append(u)
            carry = attn_block(1, 1, inj11, carry)

            # ---- tail ------------------------------------------------------
            for fns in carry[:3]:
                for fn in fns:
                    fn()
            normalize_piece()
            for i, st in enumerate(range(8, 12)):
                c_unit(st, 0, "act", tag=["inj", "s0"][i % 2])()
                c_unit(st, 1, "dve", tag="s1" if i % 2 else "inj")()
            normalize_piece()
            for i, st in enumerate(range(12, 16)):
                c_unit(st, 0, "act", tag=["inj", "s0"][i % 2])()
                c_unit(st, 1, "dve", tag="s1" if i % 2 else "inj")()

    nc.finalize()
    return nc


def make_in_maps(query, key, value, mask, Wq, bq, Wk, bk, Wv, bv, Wp, bp):
    """Shard + pre-layout the full inputs into 8 per-core input dicts."""
    in_maps = []
    for c in range(8):
        n = c // 4
        h0 = HPC * (c % 4)
        hs = slice(h0, h0 + HPC)

        def t_bf(x):  # [SEQ, DIN] -> contiguous [DIN, SEQ] bf16
            return np.ascontiguousarray(x.T).astype(BF_NP)

        # (H', DIN, DOUT) -> (DIN, H'*DOUT), head-major columns
        def w_bf(W):
            return np.ascontiguousarray(
                W[hs].transpose(1, 0, 2).reshape(DIN, HPC * DOUT)
            ).astype(BF_NP)

        # per-pair per-partition bias: [128, 2], col p = concat of heads (2p, 2p+1)
        def b_pair(b):
            return np.ascontiguousarray(b[hs].reshape(NPAIR, P).T).astype(np.float32)

        in_maps.append(
            {
                "xqT": t_bf(query[n]),
                "xkT": t_bf(key[n]),
                "xvT": t_bf(value[n]),
                "maskT": np.ascontiguousarray((~mask[n]).T).astype(BF_NP),
                "wq": w_bf(Wq),
                "wk": w_bf(Wk),
                "wv": w_bf(Wv),
                "wp": np.ascontiguousarray(
                    Wp[h0 * DOUT : (h0 + HPC) * DOUT, :]
                ).astype(BF_NP),
                "bqp": b_pair(bq),
                "bkp": b_pair(bk),
                "bvr": np.ascontiguousarray(
                    np.tile(bv[hs].reshape(1, HPC * DOUT), (P, 1))
                ).astype(np.float32),
            }
        )
    return in_maps


def kernel(**inputs):
    global _NC_CACHE
    from concourse.bass_utils import run_bass_kernel_spmd

    if _NC_CACHE is None:
        _NC_CACHE = build_bass()
    nc = _NC_CACHE

    in_maps = make_in_maps(**inputs)
    res = run_bass_kernel_spmd(nc, in_maps, core_ids=list(range(8))).results
    parts = [res[c]["out"].astype(np.float32) for c in range(8)]
    bp = inputs["bp"]
    out = np.stack(
        [
            parts[0] + parts[1] + parts[2] + parts[3] + bp[None, :],
            parts[4] + parts[5] + parts[6] + parts[7] + bp[None, :],
        ]
    )
    return out.astype(np.float32)
